# revision 1
# baseline (speedup 1.0000x reference)
"""GRUCell + LayerNorm readout fused Bass kernel for Trainium2 (8 NeuronCores).

Problem: B=8192, D=H=O=1024 fp32.
    r = sigmoid(x@Wir + bir + h@Whr)
    z = sigmoid(x@Wiz + biz + h@Whz)
    n = tanh(x@Win + bin_ + r*(h@Whn + bhn))
    new_h = (1-z)*n + z*h
    out = (LayerNorm(new_h)*ln_scale + ln_bias) @ Wout + bout

Strategy:
  - Data-parallel over batch: core c gets rows [c*1024, (c+1)*1024); weights
    replicated. No collectives.
  - Everything computed in the transposed domain: activations live as
    [feature, batch] so matmuls take the weights in natural [k, h] layout as
    the stationary operand and xT/hT as the moving operand, and the per-h gate
    biases become per-partition activation biases. Host passes xT/hT and
    transposes the outputs back.
  - float32r matmuls: 4x faster than fp32 on the PE at ~1.5e-4 rel error.
  - Matmuls are emitted k-major with both batch chunks interleaved so the PE
    can start as soon as the first input slices land (DMA-matched ramp); the
    8 gate accumulators occupy all 8 PSUM banks.
  - LayerNorm reduces over h (= partition dim): per-tile partials accumulate
    elementwise on GpSimd, one partition_all_reduce per stat at the end.
    The normalize-then-matmul is algebraically folded:
        LN(new_h) @ (ln_scale*Wout) + (ln_bias@Wout + bout)
      = rstd[b]*( new_h@WoutF - mu[b]*colsum[o] ) + boutF[o]
    with WoutF = ln_scale[:,None]*Wout (host), colsum = ln_scale@Wout (host),
    boutF = bout + ln_bias@Wout (host); the mu*colsum term is a K=1 rank-1
    matmul into the same PSUM accumulator.
"""

import sys
from contextlib import ExitStack

sys.path.insert(0, "/opt/trn_rl_repo")

import numpy as np

import concourse.bacc as bacc
import concourse.mybir as mybir
import concourse.tile as tile
from concourse import bass_isa, bass_utils

B, D, H, O = 8192, 1024, 1024, 1024
NCORES = 8
BL = B // NCORES          # batch rows per core
P = 128                   # partitions
KT = D // P               # contraction tiles (8)
HT = H // P               # h output-partition tiles (8)
OT = O // P               # o output-partition tiles (8)
NB = 2                    # batch chunks per core (free dim 512)
NF = BL // NB             # free dim per chunk (512)
LN_EPS = 1e-6

F32 = mybir.dt.float32
F32R = mybir.dt.float32r

_COMPILED = None  # compiled Bacc module cache across calls
TRACE = False     # set by test harness to capture an NTFF profile
LAST_RES = None   # BassKernelResults of the last run (for the test harness)

XGATES = ("ir", "iz", "in")
HGATES = ("hr", "hz", "hn")


def _build():
    nc = bacc.Bacc("TRN2", target_bir_lowering=False, debug=False,
                   num_devices=NCORES)

    def din(name, shape, dt=F32R):
        return nc.dram_tensor(name, shape, dt, kind="ExternalInput").ap()

    def dout(name, shape, dt=F32):
        return nc.dram_tensor(name, shape, dt, kind="ExternalOutput").ap()

    xT_d = din("xT", [D, BL])
    hT_d = din("hT", [H, BL])
    w_d = {g: din(f"W{g}", [D, H]) for g in XGATES + HGATES}
    woutF_d = din("woutF", [H, O])
    bir_d = din("bir", [H], F32)
    biz_d = din("biz", [H], F32)
    bin_d = din("bin", [H], F32)
    bhn_d = din("bhn", [H], F32)
    boutF_d = din("boutF", [O], F32)
    colsum_d = din("colsum", [1, O])
    ones_col_d = din("ones_col", [P, 1])
    ones_row_d = din("ones_row", [1, P])

    nhT_d = dout("nhT", [H, BL])
    outT_d = dout("outT", [O, BL])

    with tile.TileContext(nc) as tc, ExitStack() as ctx:
        singles = ctx.enter_context(tc.tile_pool(name="singles", bufs=1))
        wpool = ctx.enter_context(tc.tile_pool(name="wpool", bufs=2))
        gates = ctx.enter_context(tc.tile_pool(name="gates", bufs=1))
        rows = ctx.enter_context(tc.tile_pool(name="rows", bufs=1))
        ps = ctx.enter_context(tc.tile_pool(name="ps", bufs=1, space="PSUM"))

        # ---- resident inputs, DMA-ordered to feed the PE ramp ---------------
        def kslice_tile(prefix, k):
            return singles.tile([P, BL], F32R, tag=f"{prefix}{k}",
                                name=f"{prefix}{k}")

        def load_w(g, ht):
            t = wpool.tile([P, KT, P], F32R, tag=f"w{g}", name=f"w{g}_{ht}")
            nc.sync.dma_start(t[:], w_d[g][:, ht * P:(ht + 1) * P].rearrange(
                "(t p) h -> p t h", p=P))
            return t

        xT_sb, hT_sb = [], []
        for k in range(KT):
            xT_sb.append(kslice_tile("xk", k))
            hT_sb.append(kslice_tile("hk", k))

        # x slice 0, then the x-side weights for ht=0, then the rest of x,
        # then the h-side weights for ht=0, then h.
        nc.sync.dma_start(xT_sb[0][:], xT_d[0:P, :])
        w0 = {g: load_w(g, 0) for g in XGATES}
        for k in range(1, KT):
            nc.sync.dma_start(xT_sb[k][:], xT_d[k * P:(k + 1) * P, :])
        w0["hr"] = load_w("hr", 0)
        nc.sync.dma_start(hT_sb[0][:], hT_d[0:P, :])
        w0["hz"] = load_w("hz", 0)
        nc.sync.dma_start(hT_sb[1][:], hT_d[P:2 * P, :])
        w0["hn"] = load_w("hn", 0)
        for k in range(2, KT):
            nc.sync.dma_start(hT_sb[k][:], hT_d[k * P:(k + 1) * P, :])

        def load_vec(ap_d, n, tag):
            t = singles.tile([P, n // P], F32, tag=tag, name=tag)
            nc.sync.dma_start(t[:], ap_d.rearrange("(t p) -> p t", p=P))
            return t

        bir_sb = load_vec(bir_d, H, "bir_sb")
        biz_sb = load_vec(biz_d, H, "biz_sb")
        bin_sb = load_vec(bin_d, H, "bin_sb")
        bhn_sb = load_vec(bhn_d, H, "bhn_sb")
        boutF_sb = load_vec(boutF_d, O, "boutF_sb")
        colsum_sb = singles.tile([1, O], F32R)
        nc.sync.dma_start(colsum_sb[:], colsum_d)
        ones_col = singles.tile([P, 1], F32R)
        nc.sync.dma_start(ones_col[:], ones_col_d)
        ones_row = singles.tile([1, P], F32R)
        nc.sync.dma_start(ones_row[:], ones_row_d)
        eps_sb = singles.tile([1, 1], F32)
        nc.vector.memset(eps_sb[:], LN_EPS)

        new_hT_sb = [singles.tile([P, BL], F32R, tag=f"nh{ht}",
                                  name=f"nh{ht}") for ht in range(HT)]
        s_acc = [singles.tile([P, NF], F32R, tag=f"s_acc{bc}",
                              name=f"s_acc{bc}") for bc in range(NB)]
        q_acc = [singles.tile([P, NF], F32R, tag=f"q_acc{bc}",
                              name=f"q_acc{bc}") for bc in range(NB)]

        # ---- phase 1: gates + new_h -----------------------------------------
        woutF_sb = singles.tile([P, KT, O], F32R)

        for ht in range(HT):
            hs = slice(ht * P, (ht + 1) * P)
            w_sb = w0 if ht == 0 else {g: load_w(g, ht)
                                       for g in XGATES + HGATES}
            if ht == 2:
                # readout weights: resident; loaded after the ramp-critical
                # input/gate-weight prefetches are in flight
                nc.sync.dma_start(woutF_sb[:], woutF_d.rearrange(
                    "(t p) o -> p t o", p=P))

            pr = [ps.tile([P, NF], F32, tag=f"r{bc}", name=f"pr{bc}_{ht}")
                  for bc in range(NB)]
            pz = [ps.tile([P, NF], F32, tag=f"z{bc}", name=f"pz{bc}_{ht}")
                  for bc in range(NB)]
            pgi = [ps.tile([P, NF], F32, tag=f"gi{bc}", name=f"pgi{bc}_{ht}")
                   for bc in range(NB)]
            pgh = [ps.tile([P, NF], F32, tag=f"gh{bc}", name=f"pgh{bc}_{ht}")
                   for bc in range(NB)]

            bsl = [slice(bc * NF, (bc + 1) * NF) for bc in range(NB)]

            # k-major, both batch chunks interleaved: x side then h side.
            for k in range(KT):
                for bc in range(NB):
                    xs = xT_sb[k][:, bsl[bc]]
                    nc.tensor.matmul(pr[bc][:], w_sb["ir"][:, k, :], xs,
                                     start=(k == 0), stop=False)
                    nc.tensor.matmul(pz[bc][:], w_sb["iz"][:, k, :], xs,
                                     start=(k == 0), stop=False)
                    nc.tensor.matmul(pgi[bc][:], w_sb["in"][:, k, :], xs,
                                     start=(k == 0), stop=(k == KT - 1))
            for k in range(KT):
                for bc in range(NB):
                    hss = hT_sb[k][:, bsl[bc]]
                    nc.tensor.matmul(pr[bc][:], w_sb["hr"][:, k, :], hss,
                                     start=False, stop=(k == KT - 1))
                    nc.tensor.matmul(pz[bc][:], w_sb["hz"][:, k, :], hss,
                                     start=False, stop=(k == KT - 1))
                    nc.tensor.matmul(pgh[bc][:], w_sb["hn"][:, k, :], hss,
                                     start=(k == 0), stop=(k == KT - 1))

            for bc in range(NB):
                bs = bsl[bc]
                r_sb = gates.tile([P, NF], F32, tag="r_act")
                nc.scalar.activation(r_sb[:], pr[bc][:],
                                     mybir.ActivationFunctionType.Sigmoid,
                                     bias=bir_sb[:, ht:ht + 1])
                z_sb = gates.tile([P, NF], F32, tag="z_act")
                nc.scalar.activation(z_sb[:], pz[bc][:],
                                     mybir.ActivationFunctionType.Sigmoid,
                                     bias=biz_sb[:, ht:ht + 1])

                t_sb = gates.tile([P, NF], F32, tag="t")
                nc.vector.tensor_scalar(t_sb[:], pgh[bc][:],
                                        bhn_sb[:, ht:ht + 1],
                                        None, mybir.AluOpType.add)
                nc.vector.tensor_mul(t_sb[:], t_sb[:], r_sb[:])
                nc.vector.tensor_add(t_sb[:], t_sb[:], pgi[bc][:])
                n_sb = gates.tile([P, NF], F32, tag="r_act", name="n_sb")
                nc.scalar.activation(n_sb[:], t_sb[:],
                                     mybir.ActivationFunctionType.Tanh,
                                     bias=bin_sb[:, ht:ht + 1])

                u_sb = gates.tile([P, NF], F32, tag="u")
                nc.vector.tensor_tensor(u_sb[:], hT_sb[ht][:, bs].bitcast(F32),
                                        n_sb[:], mybir.AluOpType.subtract)
                nc.vector.tensor_mul(u_sb[:], z_sb[:], u_sb[:])
                nh = new_hT_sb[ht][:, bs]
                nc.vector.tensor_add(nh, n_sb[:], u_sb[:])
                nhf = nh.bitcast(F32)

                # LN stat partials: elementwise accumulate over h-tiles (DVE),
                # cross-partition reduce later via a ones-column matmul.
                sq_sb = gates.tile([P, NF], F32R, tag="t", name="sq_sb")
                if ht == 0:
                    nc.vector.tensor_copy(s_acc[bc][:], nhf)
                    nc.scalar.activation(q_acc[bc][:], nhf,
                                         mybir.ActivationFunctionType.Square)
                else:
                    nc.vector.tensor_tensor(s_acc[bc][:],
                                            s_acc[bc][:].bitcast(F32), nhf,
                                            mybir.AluOpType.add)
                    nc.scalar.activation(sq_sb[:], nhf,
                                         mybir.ActivationFunctionType.Square)
                    nc.vector.tensor_tensor(q_acc[bc][:],
                                            q_acc[bc][:].bitcast(F32),
                                            sq_sb[:].bitcast(F32),
                                            mybir.AluOpType.add)

                # stores go through GpSimd's DMA queue so they never
                # head-of-line-block weight loads on the Sync queue
                nc.gpsimd.dma_start(nhT_d[hs, bs], nhf)

        # ---- phase 2: LN scale factors + readout ----------------------------
        # bc=0 groups run first so the bc=1 stats chain hides under them; the
        # stats reduce-matmuls, broadcast matmuls, rank-1s, and epilogues are
        # all software-pipelined into the main matmul stream so the in-order
        # PE never stalls on the stats chain or cools down (HAM).
        red_tags = ("gi0", "gi1", "gh0", "gh1")
        nmu_row = {}
        rstd_row = {}
        rstd_bc = {}

        def emit_stats(bc):
            psum_s = ps.tile([1, NF], F32, tag=red_tags[2 * bc],
                             name=f"psum_s{bc}")
            nc.tensor.matmul(psum_s[:], ones_col[:], s_acc[bc][:],
                             start=True, stop=True)
            psum_q = ps.tile([1, NF], F32, tag=red_tags[2 * bc + 1],
                             name=f"psum_q{bc}")
            nc.tensor.matmul(psum_q[:], ones_col[:], q_acc[bc][:],
                             start=True, stop=True)

            nmu = rows.tile([1, NF], F32R, tag=f"nmu{bc}", name=f"nmu{bc}")
            nc.vector.tensor_scalar_mul(nmu[:], psum_s[:], -1.0 / H)
            nmu_row[bc] = nmu

            mu2 = gates.tile([1, NF], F32, tag="t", name=f"mu2_{bc}")
            nc.vector.tensor_mul(mu2[:], nmu[:].bitcast(F32), nmu[:].bitcast(F32))
            var = gates.tile([1, NF], F32, tag="u", name=f"var_{bc}")
            nc.vector.tensor_scalar_mul(var[:], psum_q[:], 1.0 / H)
            nc.vector.tensor_tensor(var[:], var[:], mu2[:],
                                    mybir.AluOpType.subtract)
            nc.scalar.activation(var[:], var[:],
                                 mybir.ActivationFunctionType.Sqrt,
                                 bias=eps_sb[:])
            rrow = gates.tile([1, NF], F32R, tag=("z_act", "r_act")[bc],
                              name=f"rstd{bc}")
            with nc.allow_low_precision(reason="f32r is fp32-width"):
                nc.vector.reciprocal(rrow[:], var[:])
            rstd_row[bc] = rrow

        po_tags = ("r0", "z0", "r1", "z1", "gh0", "gh1")
        PIPE = 5
        groups = [(ot, bc) for bc in range(NB) for ot in range(OT)]
        pending = {}

        def finalize(i):
            ot, bc = groups[i]
            po = pending.pop(i)
            os_ = slice(ot * P, (ot + 1) * P)
            bs = slice(bc * NF, (bc + 1) * NF)
            # -= mu[b] * colsum[o]  (rank-1, K=1)
            nc.tensor.matmul(po[:], colsum_sb[0:1, os_], nmu_row[bc][:],
                             start=False, stop=True)
            o_sb = gates.tile([P, NF], F32, tag=("t", "u", "z_act")[i % 3],
                              name=f"o_{ot}_{bc}")
            nc.vector.tensor_mul(o_sb[:], po[:], rstd_bc[bc][:])
            nc.vector.tensor_scalar(o_sb[:], o_sb[:],
                                    boutF_sb[:, ot:ot + 1], None,
                                    mybir.AluOpType.add)
            nc.gpsimd.dma_start(outT_d[os_, bs], o_sb[:])

        def emit_pb(bc):
            pb = ps.tile([P, NF], F32, tag=red_tags[bc], name=f"pb{bc}")
            nc.tensor.matmul(pb[:], ones_row[:], rstd_row[bc][:],
                             start=True, stop=True)
            rb = rows.tile([P, NF], F32, tag=f"rstd_bc{bc}",
                           name=f"rstd_bc{bc}")
            nc.vector.tensor_copy(rb[:], pb[:])
            rstd_bc[bc] = rb

        done = 0
        for i, (ot, bc) in enumerate(groups):
            bs = slice(bc * NF, (bc + 1) * NF)
            po = ps.tile([P, NF], F32, tag=po_tags[i % len(po_tags)],
                         name=f"po_{ot}_{bc}")
            for k in range(HT):
                nc.tensor.matmul(po[:], woutF_sb[:, k, ot * P:(ot + 1) * P],
                                 new_hT_sb[k][:, bs],
                                 start=(k == 0), stop=False)
            pending[i] = po
            if i == 0:
                emit_stats(0)
            elif i == 1:
                emit_stats(1)
            elif i == 3:
                emit_pb(0)
            elif i == 7:
                emit_pb(1)
            if i >= PIPE:
                finalize(done)
                done += 1
            if i >= 8 and done <= i - 1:
                # drain the pipeline early so the tail is short
                finalize(done)
                done += 1
        while done < len(groups):
            finalize(done)
            done += 1

    nc.compile()
    return nc


def kernel(x, h, Wir, bir, Wiz, biz, Win, bin_, Whr, Whz, Whn, bhn,
           ln_scale, ln_bias, Wout, bout):
    global _COMPILED, LAST_RES
    if _COMPILED is None:
        _COMPILED = _build()
    nc = _COMPILED

    x = np.asarray(x, np.float32)
    h = np.asarray(h, np.float32)
    xT = np.ascontiguousarray(x.T)
    hT = np.ascontiguousarray(h.T)
    Wout = np.asarray(Wout, np.float32)
    ln_scale = np.asarray(ln_scale, np.float32)
    ln_bias = np.asarray(ln_bias, np.float32)
    woutF = np.ascontiguousarray(ln_scale[:, None] * Wout)
    boutF = np.asarray(bout, np.float32) + ln_bias @ Wout
    colsum = (ln_scale @ Wout).reshape(1, O)

    common = {
        "Wir": np.asarray(Wir, np.float32), "Wiz": np.asarray(Wiz, np.float32),
        "Win": np.asarray(Win, np.float32), "Whr": np.asarray(Whr, np.float32),
        "Whz": np.asarray(Whz, np.float32), "Whn": np.asarray(Whn, np.float32),
        "woutF": woutF,
        "bir": np.asarray(bir, np.float32), "biz": np.asarray(biz, np.float32),
        "bin": np.asarray(bin_, np.float32), "bhn": np.asarray(bhn, np.float32),
        "boutF": boutF.astype(np.float32), "colsum": colsum.astype(np.float32),
        "ones_col": np.ones((P, 1), np.float32),
        "ones_row": np.ones((1, P), np.float32),
    }
    in_maps = []
    for c in range(NCORES):
        bsl = slice(c * BL, (c + 1) * BL)
        in_maps.append({
            **common,
            "xT": np.ascontiguousarray(xT[:, bsl]),
            "hT": np.ascontiguousarray(hT[:, bsl]),
        })

    res = bass_utils.run_bass_kernel_spmd(nc, in_maps,
                                          core_ids=list(range(NCORES)),
                                          trace=TRACE)
    LAST_RES = res
    new_hT = np.concatenate([res.results[c]["nhT"] for c in range(NCORES)],
                            axis=1)
    outT = np.concatenate([res.results[c]["outT"] for c in range(NCORES)],
                          axis=1)
    new_h = np.ascontiguousarray(new_hT.T)
    out = np.ascontiguousarray(outT.T)
    return new_h, out



# revision 10
# speedup vs baseline: 1.0144x; 1.0144x over previous
"""GRUCell + LayerNorm readout fused Bass kernel for Trainium2 (8 NeuronCores).

Problem: B=8192, D=H=O=1024 fp32.
    r = sigmoid(x@Wir + bir + h@Whr)
    z = sigmoid(x@Wiz + biz + h@Whz)
    n = tanh(x@Win + bin_ + r*(h@Whn + bhn))
    new_h = (1-z)*n + z*h
    out = (LayerNorm(new_h)*ln_scale + ln_bias) @ Wout + bout

Strategy (v2):
  - Data-parallel over batch: core c gets rows [c*1024, (c+1)*1024); weights
    replicated, SBUF-resident in bf16 (loaded once, used for both batch
    chunks). No collectives.
  - Transposed domain: activations live as [feature, batch]; weights are the
    stationary operand in natural [k, h] layout; per-h gate biases become
    per-partition activation biases.
  - All matmul operands bf16 (measured end-to-end rel err ~7e-3 vs the 2e-2
    gate); PSUM + epilogue arithmetic fp32. Host pre-packs weights/x/h into
    the exact SBUF layouts so every DMA is 128 descriptors of contiguous
    >=1KB lines (enqueue- and HBM-efficient).
  - HAM pre-warm: a run of dummy matmuls on a memset tile at kernel start
    flips the PE clock gate to 8/8 before the first real matmul arrives.
  - Batch-split phasing hides the gates->readout boundary: phase A = gates
    for batch chunk 0; phase B = gates for chunk 1 interleaved with the
    readout for chunk 0; phase C = readout for chunk 1. The PE never waits
    on an epilogue chain.
  - LayerNorm folded into the readout:
        out = rstd[b]*( new_h@WoutF - mu[b]*colsum[o] + boutF[o]*sd[b] )
      with WoutF = ln_scale[:,None]*Wout, colsum = ln_scale@Wout,
      boutF = bout + ln_bias@Wout, sd[b] = sqrt(var+eps) = 1/rstd[b].
    The correction is a single K=2 rank-2 matmul into the same PSUM
    accumulator (stationary = [colsum; boutF], moving = [-mu; sd]), so the
    epilogue per readout tile is ONE vector op: out = po * rstd_bcast.
  - LN stats: per-tile elementwise accumulation of sum / sum-of-squares on
    DVE, one ones-column matmul per stat to reduce over h (partition dim),
    rstd broadcast back over partitions with a ones-row matmul.
"""

import sys
from contextlib import ExitStack

sys.path.insert(0, "/opt/trn_rl_repo")

import ml_dtypes
import numpy as np

import concourse.bacc as bacc
import concourse.mybir as mybir
import concourse.tile as tile
from concourse import bass_utils

B, D, H, O = 8192, 1024, 1024, 1024
NCORES = 8
BL = B // NCORES          # batch rows per core
P = 128                   # partitions
KT = D // P               # contraction tiles (8)
HT = H // P               # h output-partition tiles (8)
OT = O // P               # o output-partition tiles (8)
NB = 2                    # batch chunks per core
NF = BL // NB             # free dim per chunk (512)
LN_EPS = 1e-6
N_WARM = 56               # HAM pre-warm dummy matmuls

F32 = mybir.dt.float32
F32R = mybir.dt.float32r
BF16 = mybir.dt.bfloat16
BF16_NP = ml_dtypes.bfloat16

_COMPILED = None
TRACE = False
LAST_RES = None

XGATES = ("ir", "iz", "in")
HGATES = ("hr", "hz", "hn")
ALLGATES = XGATES + HGATES


def _build():
    nc = bacc.Bacc("TRN2", target_bir_lowering=False, debug=False,
                   num_devices=NCORES)
    sig = mybir.ActivationFunctionType.Sigmoid
    tanh = mybir.ActivationFunctionType.Tanh
    square = mybir.ActivationFunctionType.Square
    sqrtf = mybir.ActivationFunctionType.Sqrt
    copyf = mybir.ActivationFunctionType.Copy
    add_op = mybir.AluOpType.add
    sub_op = mybir.AluOpType.subtract
    mul_op = mybir.AluOpType.mult

    def din(name, shape, dt=BF16):
        return nc.dram_tensor(name, shape, dt, kind="ExternalInput").ap()

    def dout(name, shape, dt=BF16):
        return nc.dram_tensor(name, shape, dt, kind="ExternalOutput").ap()

    # host-pre-packed inputs (see kernel() for the exact layouts)
    x_d = [din(f"x{bc}", [P, KT, NF]) for bc in range(NB)]
    h_d = [din(f"h{bc}", [P, KT, NF]) for bc in range(NB)]
    w_d = {g: din(f"W{g}", [P, HT, KT * P]) for g in ALLGATES}
    woutF_d = din("woutF", [P, KT, O])
    colsum2_d = din("colsum2", [2, O])
    ones_row_d = din("ones_row", [1, P])
    ones_col_d = din("ones_col", [P, 1], F32R)
    bias_d = {v: din(v, [P, HT], F32) for v in ("bir", "biz", "bin", "bhn")}

    nhT_d = dout("nhT", [H, BL])
    outT_d = dout("outT", [O, BL])

    with tile.TileContext(nc) as tc, ExitStack() as ctx:
        singles = ctx.enter_context(tc.tile_pool(name="singles", bufs=1))
        gates = ctx.enter_context(tc.tile_pool(name="gates", bufs=1))
        ps = ctx.enter_context(tc.tile_pool(name="ps", bufs=1, space="PSUM"))

        # ---- HAM pre-warm: junk matmuls on a memset tile -------------------
        warm_sb = singles.tile([P, 64], BF16, name="warm_sb")
        nc.vector.memset(warm_sb[:], 0.0)
        eps_sb = singles.tile([1, 1], F32, name="eps_sb")
        nc.vector.memset(eps_sb[:], LN_EPS)
        pw = ps.tile([64, 64], F32, tag="r1", name="pw")
        for i in range(N_WARM):
            nc.tensor.matmul(pw[:], warm_sb[:], warm_sb[:],
                             start=True, stop=True)

        # ---- resident inputs, DMA-ordered to feed the PE ramp --------------
        x_sb = [singles.tile([P, KT, NF], BF16, name=f"x_sb{bc}")
                for bc in range(NB)]
        h_sb = [singles.tile([P, KT, NF], BF16, name=f"h_sb{bc}")
                for bc in range(NB)]
        w_sb = {g: singles.tile([P, HT, KT * P], BF16, name=f"w_{g}")
                for g in ALLGATES}
        woutF_sb = singles.tile([P, KT, O], BF16, name="woutF_sb")
        colsum2_sb = singles.tile([2, O], BF16, name="colsum2_sb")
        ones_row = singles.tile([1, P], BF16, name="ones_row")
        ones_col = singles.tile([P, 1], F32R, name="ones_col")
        bias_sb = {v: singles.tile([P, HT], F32, name=f"{v}_sb")
                   for v in ("bir", "biz", "bin", "bhn")}

        def load_w(g, ht):
            nc.sync.dma_start(w_sb[g][:, ht], w_d[g][:, ht])

        # supply order: ramp-critical first
        nc.sync.dma_start(x_sb[0][:, 0:2], x_d[0][:, 0:2])
        for g in XGATES:
            load_w(g, 0)
        nc.sync.dma_start(x_sb[0][:, 2:8], x_d[0][:, 2:8])
        for g in HGATES:
            load_w(g, 0)
        nc.sync.dma_start(h_sb[0][:, 0:4], h_d[0][:, 0:4])
        nc.sync.dma_start(h_sb[0][:, 4:8], h_d[0][:, 4:8])
        for v in ("bir", "biz", "bin", "bhn"):
            nc.sync.dma_start(bias_sb[v][:], bias_d[v])
        for g in ALLGATES:
            load_w(g, 1)
        for g in ALLGATES:
            load_w(g, 2)
        for g in ALLGATES:
            load_w(g, 3)
        nc.sync.dma_start(x_sb[1][:], x_d[1])
        nc.sync.dma_start(h_sb[1][:], h_d[1])
        for ht in range(4, HT):
            for g in ALLGATES:
                load_w(g, ht)
        nc.sync.dma_start(woutF_sb[:], woutF_d)
        nc.sync.dma_start(colsum2_sb[:], colsum2_d)
        nc.sync.dma_start(ones_row[:], ones_row_d)
        nc.sync.dma_start(ones_col[:], ones_col_d)

        # ---- persistent activations ---------------------------------------
        nh_sb = singles.tile([P, HT, BL], BF16, name="nh_sb")
        s_acc = [singles.tile([P, NF], F32R, name=f"s_acc{bc}")
                 for bc in range(NB)]
        q_acc = [singles.tile([P, NF], F32R, name=f"q_acc{bc}")
                 for bc in range(NB)]
        # [-mu ; sd] moving operand for the readout correction matmul
        mv = [singles.tile([2, NF], BF16, name=f"mv{bc}") for bc in range(NB)]
        rstd_f32 = [singles.tile([1, NF], F32, name=f"rstd_f32_{bc}")
                    for bc in range(NB)]
        rstd_row = [singles.tile([1, NF], BF16, name=f"rstd_row{bc}")
                    for bc in range(NB)]

        bsl = [slice(bc * NF, (bc + 1) * NF) for bc in range(NB)]

        # ---- gate group: 48 matmuls + epilogue ----------------------------
        gate_tags = {0: ("r0", "z0", "gi0", "gh0"), 1: ("r1", "z1", "gi1", "gh1")}

        def emit_gate_mms(ht, bc):
            tr, tz, tgi, tgh = gate_tags[bc]
            pr = ps.tile([P, NF], F32, tag=tr, name=f"pr{bc}_{ht}")
            pz = ps.tile([P, NF], F32, tag=tz, name=f"pz{bc}_{ht}")
            pgi = ps.tile([P, NF], F32, tag=tgi, name=f"pgi{bc}_{ht}")
            pgh = ps.tile([P, NF], F32, tag=tgh, name=f"pgh{bc}_{ht}")
            hs = slice(ht * P, (ht + 1) * P)
            for k in range(KT):
                ks = slice(k * P, (k + 1) * P)
                xs = x_sb[bc][:, k, :]
                nc.tensor.matmul(pr[:], w_sb["ir"][:, ht, ks], xs,
                                 start=(k == 0), stop=False)
                nc.tensor.matmul(pz[:], w_sb["iz"][:, ht, ks], xs,
                                 start=(k == 0), stop=False)
                nc.tensor.matmul(pgi[:], w_sb["in"][:, ht, ks], xs,
                                 start=(k == 0), stop=(k == KT - 1))
            for k in range(KT):
                ks = slice(k * P, (k + 1) * P)
                hss = h_sb[bc][:, k, :]
                nc.tensor.matmul(pr[:], w_sb["hr"][:, ht, ks], hss,
                                 start=False, stop=(k == KT - 1))
                nc.tensor.matmul(pz[:], w_sb["hz"][:, ht, ks], hss,
                                 start=False, stop=(k == KT - 1))
                nc.tensor.matmul(pgh[:], w_sb["hn"][:, ht, ks], hss,
                                 start=(k == 0), stop=(k == KT - 1))
            return pr, pz, pgi, pgh

        def emit_gate_epilogue(ht, bc, pr, pz, pgi, pgh):
            hs = slice(ht * P, (ht + 1) * P)
            bs = bsl[bc]
            r_sb = gates.tile([P, NF], F32, tag="r_act", name=f"r_{ht}_{bc}")
            nc.scalar.activation(r_sb[:], pr[:], sig,
                                 bias=bias_sb["bir"][:, ht:ht + 1])
            z_sb = gates.tile([P, NF], F32, tag="z_act", name=f"z_{ht}_{bc}")
            nc.scalar.activation(z_sb[:], pz[:], sig,
                                 bias=bias_sb["biz"][:, ht:ht + 1])
            # h upcast for the blend (exact; off the DVE critical path)
            hf = gates.tile([P, NF], F32, tag=f"hf{ht % 2}", name=f"hf_{ht}_{bc}")
            nc.scalar.activation(hf[:], h_sb[bc][:, ht, :], copyf)

            # t = (pgh + bhn) * r ; t2 = (pgi + bin) + t ; n = tanh(t2)
            t_sb = gates.tile([P, NF], F32, tag="t", name=f"t_{ht}_{bc}")
            nc.vector.scalar_tensor_tensor(
                t_sb[:], pgh[:], bias_sb["bhn"][:, ht:ht + 1], r_sb[:],
                add_op, mul_op)
            t2_sb = gates.tile([P, NF], F32, tag="u", name=f"t2_{ht}_{bc}")
            nc.vector.scalar_tensor_tensor(
                t2_sb[:], pgi[:], bias_sb["bin"][:, ht:ht + 1], t_sb[:],
                add_op, add_op)
            n_sb = gates.tile([P, NF], F32, tag="r_act", name=f"n_{ht}_{bc}")
            nc.scalar.activation(n_sb[:], t2_sb[:], tanh)

            # new_h = n + z*(h - n)
            u_sb = gates.tile([P, NF], F32, tag="t", name=f"u_{ht}_{bc}")
            nc.vector.tensor_tensor(u_sb[:], hf[:], n_sb[:], sub_op)
            v_sb = gates.tile([P, NF], F32, tag="v", name=f"v_{ht}_{bc}")
            nc.vector.tensor_mul(v_sb[:], z_sb[:], u_sb[:])
            nhf = gates.tile([P, NF], F32, tag="u", name=f"nhf_{ht}_{bc}")
            nc.vector.tensor_add(nhf[:], n_sb[:], v_sb[:])

            # LN stat partials (f32 accumulate over ht)
            if ht == 0:
                nc.vector.tensor_copy(s_acc[bc][:], nhf[:])
                nc.scalar.activation(q_acc[bc][:], nhf[:], square)
            else:
                nc.vector.tensor_tensor(s_acc[bc][:], s_acc[bc][:].bitcast(F32),
                                        nhf[:], add_op)
                sq = gates.tile([P, NF], F32, tag="t", name=f"sq_{ht}_{bc}")
                nc.scalar.activation(sq[:], nhf[:], square)
                nc.vector.tensor_tensor(q_acc[bc][:], q_acc[bc][:].bitcast(F32),
                                        sq[:], add_op)

            # bf16 copy feeds the readout matmul + the nhT store
            nc.scalar.activation(nh_sb[:, ht, bs], nhf[:], copyf)
            nc.gpsimd.dma_start(nhT_d[hs, bs], nh_sb[:, ht, bs])

        def emit_gate_group(ht, bc):
            emit_gate_epilogue(ht, bc, *emit_gate_mms(ht, bc))

        # ---- LN stats: reduce + scale-factor chain ------------------------
        st_tags = {0: "gh0", 1: "z0"}
        pb_tags = {0: "r0", 1: "gi0"}
        pb_ps = {}

        def emit_stat_mms(bc):
            # matmul PSUM dsts must start at partition 0 -> separate banks
            st_s = ps.tile([1, NF], F32, tag=st_tags[bc], name=f"st_s{bc}")
            nc.tensor.matmul(st_s[:], ones_col[:], s_acc[bc][:],
                             start=True, stop=True)
            st_q = ps.tile([1, NF], F32, tag=pb_tags[bc], name=f"st_q{bc}")
            nc.tensor.matmul(st_q[:], ones_col[:], q_acc[bc][:],
                             start=True, stop=True)
            return st_s, st_q

        def emit_stat_chain(bc, st):
            st_s, st_q = st
            # mv[0] = -mu (bf16) ; also f32 for mu^2
            nmu_f = gates.tile([1, NF], F32, tag="row0", name=f"nmu_f{bc}")
            nc.vector.tensor_scalar_mul(nmu_f[:], st_s[:], -1.0 / H)
            nc.vector.tensor_copy(mv[bc][0:1, :], nmu_f[:])
            mu2 = gates.tile([1, NF], F32, tag="row1", name=f"mu2_{bc}")
            nc.vector.tensor_mul(mu2[:], nmu_f[:], nmu_f[:])
            var = gates.tile([1, NF], F32, tag="row0", name=f"var_{bc}")
            nc.vector.scalar_tensor_tensor(var[:], st_q[:], 1.0 / H,
                                           mu2[:], mul_op, sub_op)
            # sd = sqrt(var + eps) -> mv[1] (bf16) and f32 for reciprocal
            sd_f = gates.tile([1, NF], F32, tag="row1", name=f"sd_f{bc}")
            nc.scalar.activation(sd_f[:], var[:], sqrtf, bias=eps_sb[:])
            # compute engines can't target partition 1; DMA the sd row there
            sd_bf = gates.tile([1, NF], BF16, tag="row2", name=f"sd_bf{bc}")
            nc.scalar.activation(sd_bf[:], sd_f[:], copyf)
            nc.gpsimd.dma_start(mv[bc][1:2, :], sd_bf[:])
            nc.vector.reciprocal(rstd_f32[bc][:], sd_f[:])
            nc.scalar.activation(rstd_row[bc][:], rstd_f32[bc][:], copyf)

        rstd_bc = [singles.tile([P, NF], F32, name=f"rstd_bc{bc}")
                   for bc in range(NB)]

        def emit_pb(bc):
            # DVE can read only one PSUM operand -> land the broadcast in SBUF
            pb = ps.tile([P, NF], F32, tag=pb_tags[bc], name=f"pb{bc}")
            nc.tensor.matmul(pb[:], ones_row[:], rstd_row[bc][:],
                             start=True, stop=True)
            nc.vector.tensor_copy(rstd_bc[bc][:], pb[:])
            pb_ps[bc] = rstd_bc[bc]

        # ---- readout group: 8 k-matmuls + rank-2 correction + 1 DVE op ----
        po_tags = {0: ("z0", "gi0"), 1: ("r1", "z1", "gi1", "gh1")}

        def emit_readout_mms(ot, bc):
            tags = po_tags[bc]
            po = ps.tile([P, NF], F32, tag=tags[ot % len(tags)],
                         name=f"po_{ot}_{bc}")
            os_ = slice(ot * P, (ot + 1) * P)
            bs = bsl[bc]
            for k in range(HT):
                nc.tensor.matmul(po[:], woutF_sb[:, k, os_],
                                 nh_sb[:, k, bs],
                                 start=(k == 0), stop=False)
            return po

        def emit_readout_corr(ot, bc, po):
            os_ = slice(ot * P, (ot + 1) * P)
            nc.tensor.matmul(po[:], colsum2_sb[:, os_], mv[bc][:],
                             start=False, stop=True)

        def emit_readout_fin(ot, bc, po):
            os_ = slice(ot * P, (ot + 1) * P)
            bs = bsl[bc]
            o_sb = gates.tile([P, NF], BF16, tag=f"o{ot % 3}",
                              name=f"o_{ot}_{bc}")
            nc.vector.tensor_mul(o_sb[:], po[:], pb_ps[bc][:])
            nc.gpsimd.dma_start(outT_d[os_, bs], o_sb[:])

        def emit_readout(ot, bc):
            po = emit_readout_mms(ot, bc)
            emit_readout_corr(ot, bc, po)
            emit_readout_fin(ot, bc, po)
            return po

        # ---- phase A: gates bc0 -------------------------------------------
        for ht in range(HT):
            emit_gate_group(ht, 0)

        # ---- phase B: gates bc1 + readout bc0 -----------------------------
        emit_gate_group(0, 1)
        st0 = emit_stat_mms(0)
        emit_stat_chain(0, st0)
        emit_gate_group(1, 1)
        po0 = emit_readout_mms(0, 0)
        emit_readout_corr(0, 0, po0)
        emit_gate_mms_out = emit_gate_mms(2, 1)
        emit_pb(0)
        emit_readout_fin(0, 0, po0)
        emit_gate_epilogue(2, 1, *emit_gate_mms_out)
        emit_readout(1, 0)
        for ht in range(3, HT):
            emit_gate_group(ht, 1)
            emit_readout(ht - 1, 0)
        emit_readout(7, 0)

        # ---- phase C: readout bc1 -----------------------------------------
        st1 = emit_stat_mms(1)
        emit_stat_chain(1, st1)
        pos = {}
        for ot in range(OT):
            pos[ot] = emit_readout_mms(ot, 1)
            if ot == 2:
                emit_pb(1)
            if ot >= 2:
                emit_readout_corr(ot - 2, 1, pos[ot - 2])
                emit_readout_fin(ot - 2, 1, pos.pop(ot - 2))
        for ot in (6, 7):
            emit_readout_corr(ot, 1, pos[ot])
            emit_readout_fin(ot, 1, pos.pop(ot))

    nc.compile()
    return nc


def _pack_weight(w):
    # [D, H] -> [P, HT, KT*P] with [p, ht, k*P+j] = w[k*P+p, ht*P+j]
    t = np.asarray(w, np.float32).reshape(KT, P, HT, P)
    return np.ascontiguousarray(
        t.transpose(1, 2, 0, 3).reshape(P, HT, KT * P).astype(BF16_NP))


def kernel(x, h, Wir, bir, Wiz, biz, Win, bin_, Whr, Whz, Whn, bhn,
           ln_scale, ln_bias, Wout, bout):
    global _COMPILED, LAST_RES
    if _COMPILED is None:
        _COMPILED = _build()
    nc = _COMPILED

    ln_scale = np.asarray(ln_scale, np.float32)
    ln_bias = np.asarray(ln_bias, np.float32)
    Wout = np.asarray(Wout, np.float32)
    woutF = ln_scale[:, None] * Wout
    woutF_p = np.ascontiguousarray(
        woutF.reshape(KT, P, O).transpose(1, 0, 2).astype(BF16_NP))
    boutF = np.asarray(bout, np.float32) + ln_bias @ Wout
    colsum = ln_scale @ Wout
    colsum2 = np.ascontiguousarray(
        np.stack([colsum, boutF]).astype(BF16_NP))

    def pack_vec(v):
        return np.ascontiguousarray(
            np.asarray(v, np.float32).reshape(HT, P).T)

    common = {
        "Wir": _pack_weight(Wir), "Wiz": _pack_weight(Wiz),
        "Win": _pack_weight(Win), "Whr": _pack_weight(Whr),
        "Whz": _pack_weight(Whz), "Whn": _pack_weight(Whn),
        "woutF": woutF_p, "colsum2": colsum2,
        "bir": pack_vec(bir), "biz": pack_vec(biz),
        "bin": pack_vec(bin_), "bhn": pack_vec(bhn),
        "ones_row": np.ones((1, P), BF16_NP),
        "ones_col": np.ones((P, 1), np.float32),
    }

    def pack_act(a, rows):
        # [BL, D] slice -> per-bc [P, KT, NF] with [p, k, f] = a[bc*NF+f, k*P+p]
        arr = np.asarray(a, np.float32)[rows].T.reshape(KT, P, NB, NF)
        arr = arr.transpose(1, 0, 2, 3).astype(BF16_NP)
        return [np.ascontiguousarray(arr[:, :, bc, :]) for bc in range(NB)]

    in_maps = []
    for c in range(NCORES):
        rows = slice(c * BL, (c + 1) * BL)
        xp = pack_act(x, rows)
        hp = pack_act(h, rows)
        in_maps.append({
            **common,
            "x0": xp[0], "x1": xp[1], "h0": hp[0], "h1": hp[1],
        })

    res = bass_utils.run_bass_kernel_spmd(nc, in_maps,
                                          core_ids=list(range(NCORES)),
                                          trace=TRACE)
    LAST_RES = res
    new_hT = np.concatenate(
        [res.results[c]["nhT"].astype(np.float32) for c in range(NCORES)],
        axis=1)
    outT = np.concatenate(
        [res.results[c]["outT"].astype(np.float32) for c in range(NCORES)],
        axis=1)
    return np.ascontiguousarray(new_hT.T), np.ascontiguousarray(outT.T)


# revision 13
# speedup vs baseline: 1.1984x; 1.1815x over previous
"""GRUCell + LayerNorm readout fused Bass kernel for Trainium2 (8 NeuronCores).

Problem: B=8192, D=H=O=1024 fp32.
    r = sigmoid(x@Wir + bir + h@Whr)
    z = sigmoid(x@Wiz + biz + h@Whz)
    n = tanh(x@Win + bin_ + r*(h@Whn + bhn))
    new_h = (1-z)*n + z*h
    out = (LayerNorm(new_h)*ln_scale + ln_bias) @ Wout + bout

Strategy (v2):
  - Data-parallel over batch: core c gets rows [c*1024, (c+1)*1024); weights
    replicated, SBUF-resident in bf16 (loaded once, used for both batch
    chunks). No collectives.
  - Transposed domain: activations live as [feature, batch]; weights are the
    stationary operand in natural [k, h] layout; per-h gate biases become
    per-partition activation biases.
  - All matmul operands bf16 (measured end-to-end rel err ~7e-3 vs the 2e-2
    gate); PSUM + epilogue arithmetic fp32. Host pre-packs weights/x/h into
    the exact SBUF layouts so every DMA is 128 descriptors of contiguous
    >=1KB lines (enqueue- and HBM-efficient).
  - HAM pre-warm: a run of dummy matmuls on a memset tile at kernel start
    flips the PE clock gate to 8/8 before the first real matmul arrives.
  - Batch-split phasing hides the gates->readout boundary: phase A = gates
    for batch chunk 0; phase B = gates for chunk 1 interleaved with the
    readout for chunk 0; phase C = readout for chunk 1. The PE never waits
    on an epilogue chain.
  - LayerNorm folded into the readout:
        out = rstd[b]*( new_h@WoutF - mu[b]*colsum[o] + boutF[o]*sd[b] )
      with WoutF = ln_scale[:,None]*Wout, colsum = ln_scale@Wout,
      boutF = bout + ln_bias@Wout, sd[b] = sqrt(var+eps) = 1/rstd[b].
    The correction is a single K=2 rank-2 matmul into the same PSUM
    accumulator (stationary = [colsum; boutF], moving = [-mu; sd]), so the
    epilogue per readout tile is ONE vector op: out = po * rstd_bcast.
  - LN stats: per-tile elementwise accumulation of sum / sum-of-squares on
    DVE, one ones-column matmul per stat to reduce over h (partition dim),
    rstd broadcast back over partitions with a ones-row matmul.
"""

import sys
from contextlib import ExitStack

sys.path.insert(0, "/opt/trn_rl_repo")

import ml_dtypes
import numpy as np

import concourse.bacc as bacc
import concourse.mybir as mybir
import concourse.tile as tile
from concourse import bass_utils

B, D, H, O = 8192, 1024, 1024, 1024
NCORES = 8
BL = B // NCORES          # batch rows per core
P = 128                   # partitions
KT = D // P               # contraction tiles (8)
HT = H // P               # h output-partition tiles (8)
OT = O // P               # o output-partition tiles (8)
NB = 2                    # batch chunks per core
NF = BL // NB             # free dim per chunk (512)
LN_EPS = 1e-6
N_WARM = 56               # HAM pre-warm dummy matmuls

F32 = mybir.dt.float32
F32R = mybir.dt.float32r
BF16 = mybir.dt.bfloat16
BF16_NP = ml_dtypes.bfloat16

_COMPILED = None
TRACE = False
LAST_RES = None

XGATES = ("ir", "iz", "in")
HGATES = ("hr", "hz", "hn")
ALLGATES = XGATES + HGATES


def _build():
    nc = bacc.Bacc("TRN2", target_bir_lowering=False, debug=False,
                   num_devices=NCORES)
    sig = mybir.ActivationFunctionType.Sigmoid
    tanh = mybir.ActivationFunctionType.Tanh
    square = mybir.ActivationFunctionType.Square
    sqrtf = mybir.ActivationFunctionType.Sqrt
    copyf = mybir.ActivationFunctionType.Copy
    add_op = mybir.AluOpType.add
    sub_op = mybir.AluOpType.subtract
    mul_op = mybir.AluOpType.mult

    def din(name, shape, dt=BF16):
        return nc.dram_tensor(name, shape, dt, kind="ExternalInput").ap()

    def dout(name, shape, dt=BF16):
        return nc.dram_tensor(name, shape, dt, kind="ExternalOutput").ap()

    # host-pre-packed inputs (see kernel() for the exact layouts)
    x_d = [din(f"x{bc}", [P, KT, NF]) for bc in range(NB)]
    h_d = [din(f"h{bc}", [P, KT, NF]) for bc in range(NB)]
    w_d = {g: din(f"W{g}", [P, HT, KT * P]) for g in ALLGATES}
    woutF_d = din("woutF", [P, KT, O])
    colsum2_d = din("colsum2", [2, O])
    ones_row_d = din("ones_row", [1, P])
    ones_col_d = din("ones_col", [P, 1], F32R)
    bias_d = {v: din(v, [P, HT], F32) for v in ("bir", "biz", "bin", "bhn")}

    nhT_d = dout("nhT", [H, BL])
    outT_d = dout("outT", [O, BL])

    with tile.TileContext(nc) as tc, ExitStack() as ctx:
        singles = ctx.enter_context(tc.tile_pool(name="singles", bufs=1))
        gates = ctx.enter_context(tc.tile_pool(name="gates", bufs=1))
        ps = ctx.enter_context(tc.tile_pool(name="ps", bufs=1, space="PSUM"))

        # ---- HAM pre-warm: junk matmuls on a memset tile -------------------
        warm_sb = singles.tile([P, 64], BF16, name="warm_sb")
        nc.vector.memset(warm_sb[:], 0.0)
        eps_sb = singles.tile([1, 1], F32, name="eps_sb")
        nc.vector.memset(eps_sb[:], LN_EPS)
        pw = ps.tile([64, 64], F32, tag="r1", name="pw")
        for i in range(N_WARM):
            nc.tensor.matmul(pw[:], warm_sb[:], warm_sb[:],
                             start=True, stop=True)

        # ---- resident inputs, DMA-ordered to feed the PE ramp --------------
        x_sb = [singles.tile([P, KT, NF], BF16, name=f"x_sb{bc}")
                for bc in range(NB)]
        h_sb = [singles.tile([P, KT, NF], BF16, name=f"h_sb{bc}")
                for bc in range(NB)]
        w_sb = {g: singles.tile([P, HT, KT * P], BF16, name=f"w_{g}")
                for g in ALLGATES}
        woutF_sb = singles.tile([P, KT, O], BF16, name="woutF_sb")
        colsum2_sb = singles.tile([2, O], BF16, name="colsum2_sb")
        ones_row = singles.tile([1, P], BF16, name="ones_row")
        ones_col = singles.tile([P, 1], F32R, name="ones_col")
        bias_sb = {v: singles.tile([P, HT], F32, name=f"{v}_sb")
                   for v in ("bir", "biz", "bin", "bhn")}

        def load_w(g, ht):
            nc.sync.dma_start(w_sb[g][:, ht], w_d[g][:, ht])

        # supply order: ramp-critical first
        nc.sync.dma_start(x_sb[0][:, 0:2], x_d[0][:, 0:2])
        for g in XGATES:
            load_w(g, 0)
        nc.sync.dma_start(x_sb[0][:, 2:8], x_d[0][:, 2:8])
        for g in HGATES:
            load_w(g, 0)
        nc.sync.dma_start(h_sb[0][:, 0:4], h_d[0][:, 0:4])
        nc.sync.dma_start(h_sb[0][:, 4:8], h_d[0][:, 4:8])
        for v in ("bir", "biz", "bin", "bhn"):
            nc.sync.dma_start(bias_sb[v][:], bias_d[v])
        for g in ALLGATES:
            load_w(g, 1)
        for g in ALLGATES:
            load_w(g, 2)
        for g in ALLGATES:
            load_w(g, 3)
        nc.sync.dma_start(x_sb[1][:], x_d[1])
        nc.sync.dma_start(h_sb[1][:], h_d[1])
        for ht in range(4, HT):
            for g in ALLGATES:
                load_w(g, ht)
        nc.sync.dma_start(woutF_sb[:], woutF_d)
        nc.sync.dma_start(colsum2_sb[:], colsum2_d)
        nc.sync.dma_start(ones_row[:], ones_row_d)
        nc.sync.dma_start(ones_col[:], ones_col_d)

        # ---- persistent activations ---------------------------------------
        nh_sb = singles.tile([P, HT, BL], BF16, name="nh_sb")
        s_acc = [singles.tile([P, NF], F32R, name=f"s_acc{bc}")
                 for bc in range(NB)]
        q_acc = [singles.tile([P, NF], F32R, name=f"q_acc{bc}")
                 for bc in range(NB)]
        # [-mu ; sd] moving operand for the readout correction matmul
        mv = [singles.tile([2, NF], BF16, name=f"mv{bc}") for bc in range(NB)]
        rstd_f32 = [singles.tile([1, NF], F32, name=f"rstd_f32_{bc}")
                    for bc in range(NB)]
        rstd_row = [singles.tile([1, NF], BF16, name=f"rstd_row{bc}")
                    for bc in range(NB)]

        bsl = [slice(bc * NF, (bc + 1) * NF) for bc in range(NB)]

        # ---- gate group: 48 matmuls + epilogue ----------------------------
        gate_tags = {0: ("r0", "z0", "gi0", "gh0"), 1: ("r1", "z1", "gi1", "gh1")}

        def emit_gate_mms(ht, bc):
            tr, tz, tgi, tgh = gate_tags[bc]
            pr = ps.tile([P, NF], F32, tag=tr, name=f"pr{bc}_{ht}")
            pz = ps.tile([P, NF], F32, tag=tz, name=f"pz{bc}_{ht}")
            pgi = ps.tile([P, NF], F32, tag=tgi, name=f"pgi{bc}_{ht}")
            pgh = ps.tile([P, NF], F32, tag=tgh, name=f"pgh{bc}_{ht}")
            hs = slice(ht * P, (ht + 1) * P)
            for k in range(KT):
                ks = slice(k * P, (k + 1) * P)
                xs = x_sb[bc][:, k, :]
                nc.tensor.matmul(pr[:], w_sb["ir"][:, ht, ks], xs,
                                 start=(k == 0), stop=False)
                nc.tensor.matmul(pz[:], w_sb["iz"][:, ht, ks], xs,
                                 start=(k == 0), stop=False)
                nc.tensor.matmul(pgi[:], w_sb["in"][:, ht, ks], xs,
                                 start=(k == 0), stop=(k == KT - 1))
            for k in range(KT):
                ks = slice(k * P, (k + 1) * P)
                hss = h_sb[bc][:, k, :]
                nc.tensor.matmul(pr[:], w_sb["hr"][:, ht, ks], hss,
                                 start=False, stop=(k == KT - 1))
                nc.tensor.matmul(pz[:], w_sb["hz"][:, ht, ks], hss,
                                 start=False, stop=(k == KT - 1))
                nc.tensor.matmul(pgh[:], w_sb["hn"][:, ht, ks], hss,
                                 start=(k == 0), stop=(k == KT - 1))
            return pr, pz, pgi, pgh

        def emit_gate_epilogue(ht, bc, pr, pz, pgi, pgh):
            hs = slice(ht * P, (ht + 1) * P)
            bs = bsl[bc]
            r_sb = gates.tile([P, NF], F32, tag="r_act", name=f"r_{ht}_{bc}")
            nc.scalar.activation(r_sb[:], pr[:], sig,
                                 bias=bias_sb["bir"][:, ht:ht + 1])
            z_sb = gates.tile([P, NF], F32, tag="z_act", name=f"z_{ht}_{bc}")
            nc.scalar.activation(z_sb[:], pz[:], sig,
                                 bias=bias_sb["biz"][:, ht:ht + 1])
            # h upcast for the blend (exact; off the DVE critical path)
            hf = gates.tile([P, NF], F32, tag=f"hf{ht % 2}", name=f"hf_{ht}_{bc}")
            nc.scalar.activation(hf[:], h_sb[bc][:, ht, :], copyf)

            # t = (pgh + bhn) * r ; t2 = (pgi + bin) + t ; n = tanh(t2)
            t_sb = gates.tile([P, NF], F32, tag="t", name=f"t_{ht}_{bc}")
            nc.vector.scalar_tensor_tensor(
                t_sb[:], pgh[:], bias_sb["bhn"][:, ht:ht + 1], r_sb[:],
                add_op, mul_op)
            t2_sb = gates.tile([P, NF], F32, tag="u", name=f"t2_{ht}_{bc}")
            nc.vector.scalar_tensor_tensor(
                t2_sb[:], pgi[:], bias_sb["bin"][:, ht:ht + 1], t_sb[:],
                add_op, add_op)
            n_sb = gates.tile([P, NF], F32, tag="r_act", name=f"n_{ht}_{bc}")
            nc.scalar.activation(n_sb[:], t2_sb[:], tanh)

            # new_h = n + z*(h - n)
            u_sb = gates.tile([P, NF], F32, tag="t", name=f"u_{ht}_{bc}")
            nc.vector.tensor_tensor(u_sb[:], hf[:], n_sb[:], sub_op)
            v_sb = gates.tile([P, NF], F32, tag="v", name=f"v_{ht}_{bc}")
            nc.vector.tensor_mul(v_sb[:], z_sb[:], u_sb[:])
            nhf = gates.tile([P, NF], F32, tag="u", name=f"nhf_{ht}_{bc}")
            nc.vector.tensor_add(nhf[:], n_sb[:], v_sb[:])

            # LN stat partials (f32 accumulate over ht)
            if ht == 0:
                nc.vector.tensor_copy(s_acc[bc][:], nhf[:])
                nc.scalar.activation(q_acc[bc][:], nhf[:], square)
            else:
                nc.vector.tensor_tensor(s_acc[bc][:], s_acc[bc][:].bitcast(F32),
                                        nhf[:], add_op)
                sq = gates.tile([P, NF], F32, tag="t", name=f"sq_{ht}_{bc}")
                nc.scalar.activation(sq[:], nhf[:], square)
                nc.vector.tensor_tensor(q_acc[bc][:], q_acc[bc][:].bitcast(F32),
                                        sq[:], add_op)

            # bf16 copy feeds the readout matmul + the nhT store
            nc.scalar.activation(nh_sb[:, ht, bs], nhf[:], copyf)
            nc.gpsimd.dma_start(nhT_d[hs, bs], nh_sb[:, ht, bs])

        def emit_gate_group(ht, bc):
            emit_gate_epilogue(ht, bc, *emit_gate_mms(ht, bc))

        # ---- LN stats: reduce + scale-factor chain ------------------------
        st_tags = {0: "gh0", 1: "z0"}
        pb_tags = {0: "r0", 1: "gi0"}
        pb_ps = {}

        def emit_stat_mms(bc):
            # matmul PSUM dsts must start at partition 0 -> separate banks
            st_s = ps.tile([1, NF], F32, tag=st_tags[bc], name=f"st_s{bc}")
            nc.tensor.matmul(st_s[:], ones_col[:], s_acc[bc][:],
                             start=True, stop=True)
            st_q = ps.tile([1, NF], F32, tag=pb_tags[bc], name=f"st_q{bc}")
            nc.tensor.matmul(st_q[:], ones_col[:], q_acc[bc][:],
                             start=True, stop=True)
            return st_s, st_q

        def emit_stat_chain(bc, st):
            st_s, st_q = st
            # mv[0] = -mu (bf16) ; also f32 for mu^2
            nmu_f = gates.tile([1, NF], F32, tag="row0", name=f"nmu_f{bc}")
            nc.vector.tensor_scalar_mul(nmu_f[:], st_s[:], -1.0 / H)
            nc.vector.tensor_copy(mv[bc][0:1, :], nmu_f[:])
            mu2 = gates.tile([1, NF], F32, tag="row1", name=f"mu2_{bc}")
            nc.vector.tensor_mul(mu2[:], nmu_f[:], nmu_f[:])
            var = gates.tile([1, NF], F32, tag="row0", name=f"var_{bc}")
            nc.vector.scalar_tensor_tensor(var[:], st_q[:], 1.0 / H,
                                           mu2[:], mul_op, sub_op)
            # sd = sqrt(var + eps) -> mv[1] (bf16) and f32 for reciprocal
            sd_f = gates.tile([1, NF], F32, tag="row1", name=f"sd_f{bc}")
            nc.scalar.activation(sd_f[:], var[:], sqrtf, bias=eps_sb[:])
            # compute engines can't target partition 1; DMA the sd row there
            sd_bf = gates.tile([1, NF], BF16, tag="row2", name=f"sd_bf{bc}")
            nc.scalar.activation(sd_bf[:], sd_f[:], copyf)
            nc.gpsimd.dma_start(mv[bc][1:2, :], sd_bf[:])
            nc.vector.reciprocal(rstd_f32[bc][:], sd_f[:])
            nc.scalar.activation(rstd_row[bc][:], rstd_f32[bc][:], copyf)

        rstd_bc = [singles.tile([P, NF], F32, name=f"rstd_bc{bc}")
                   for bc in range(NB)]

        def emit_pb(bc):
            # DVE can read only one PSUM operand -> land the broadcast in SBUF
            pb = ps.tile([P, NF], F32, tag=pb_tags[bc], name=f"pb{bc}")
            nc.tensor.matmul(pb[:], ones_row[:], rstd_row[bc][:],
                             start=True, stop=True)
            nc.vector.tensor_copy(rstd_bc[bc][:], pb[:])
            pb_ps[bc] = rstd_bc[bc]

        # ---- readout group: 8 k-matmuls + rank-2 correction + 1 DVE op ----
        po_tags = {0: ("z0", "gi0"), 1: ("r1", "z1", "gi1", "gh1")}

        def emit_readout_mms(ot, bc):
            tags = po_tags[bc]
            po = ps.tile([P, NF], F32, tag=tags[ot % len(tags)],
                         name=f"po_{ot}_{bc}")
            os_ = slice(ot * P, (ot + 1) * P)
            bs = bsl[bc]
            for k in range(HT):
                nc.tensor.matmul(po[:], woutF_sb[:, k, os_],
                                 nh_sb[:, k, bs],
                                 start=(k == 0), stop=False)
            return po

        def emit_readout_corr(ot, bc, po):
            os_ = slice(ot * P, (ot + 1) * P)
            nc.tensor.matmul(po[:], colsum2_sb[:, os_], mv[bc][:],
                             start=False, stop=True)

        def emit_readout_fin(ot, bc, po):
            os_ = slice(ot * P, (ot + 1) * P)
            bs = bsl[bc]
            o_sb = gates.tile([P, NF], BF16, tag=f"o{ot % 3}",
                              name=f"o_{ot}_{bc}")
            nc.vector.tensor_mul(o_sb[:], po[:], pb_ps[bc][:])
            nc.scalar.dma_start(outT_d[os_, bs], o_sb[:])

        def emit_readout(ot, bc):
            po = emit_readout_mms(ot, bc)
            emit_readout_corr(ot, bc, po)
            emit_readout_fin(ot, bc, po)
            return po

        # ---- phase A: gates bc0 -------------------------------------------
        for ht in range(HT):
            emit_gate_group(ht, 0)

        # ---- phase B: gates bc1 + readout bc0 -----------------------------
        emit_gate_group(0, 1)
        st0 = emit_stat_mms(0)
        emit_stat_chain(0, st0)
        emit_gate_group(1, 1)
        po0 = emit_readout_mms(0, 0)
        emit_readout_corr(0, 0, po0)
        emit_gate_mms_out = emit_gate_mms(2, 1)
        emit_pb(0)
        emit_readout_fin(0, 0, po0)
        emit_gate_epilogue(2, 1, *emit_gate_mms_out)
        emit_readout(1, 0)
        for ht in range(3, HT):
            emit_gate_group(ht, 1)
            emit_readout(ht - 1, 0)
        emit_readout(7, 0)

        # ---- phase C: readout bc1 -----------------------------------------
        st1 = emit_stat_mms(1)
        emit_stat_chain(1, st1)
        pos = {}
        for ot in range(OT):
            pos[ot] = emit_readout_mms(ot, 1)
            if ot == 2:
                emit_pb(1)
            if ot == 2:
                emit_readout_corr(0, 1, pos[0])
                emit_readout_fin(0, 1, pos.pop(0))
                emit_readout_corr(1, 1, pos[1])
                emit_readout_fin(1, 1, pos.pop(1))
            elif ot >= 3:
                emit_readout_corr(ot - 1, 1, pos[ot - 1])
                emit_readout_fin(ot - 1, 1, pos.pop(ot - 1))
        emit_readout_corr(7, 1, pos[7])
        emit_readout_fin(7, 1, pos.pop(7))

    nc.compile()
    return nc


def _pack_weight(w):
    # [D, H] -> [P, HT, KT*P] with [p, ht, k*P+j] = w[k*P+p, ht*P+j]
    t = np.asarray(w, np.float32).reshape(KT, P, HT, P)
    return np.ascontiguousarray(
        t.transpose(1, 2, 0, 3).reshape(P, HT, KT * P).astype(BF16_NP))


def kernel(x, h, Wir, bir, Wiz, biz, Win, bin_, Whr, Whz, Whn, bhn,
           ln_scale, ln_bias, Wout, bout):
    global _COMPILED, LAST_RES
    if _COMPILED is None:
        _COMPILED = _build()
    nc = _COMPILED

    ln_scale = np.asarray(ln_scale, np.float32)
    ln_bias = np.asarray(ln_bias, np.float32)
    Wout = np.asarray(Wout, np.float32)
    woutF = ln_scale[:, None] * Wout
    woutF_p = np.ascontiguousarray(
        woutF.reshape(KT, P, O).transpose(1, 0, 2).astype(BF16_NP))
    boutF = np.asarray(bout, np.float32) + ln_bias @ Wout
    colsum = ln_scale @ Wout
    colsum2 = np.ascontiguousarray(
        np.stack([colsum, boutF]).astype(BF16_NP))

    def pack_vec(v):
        return np.ascontiguousarray(
            np.asarray(v, np.float32).reshape(HT, P).T)

    common = {
        "Wir": _pack_weight(Wir), "Wiz": _pack_weight(Wiz),
        "Win": _pack_weight(Win), "Whr": _pack_weight(Whr),
        "Whz": _pack_weight(Whz), "Whn": _pack_weight(Whn),
        "woutF": woutF_p, "colsum2": colsum2,
        "bir": pack_vec(bir), "biz": pack_vec(biz),
        "bin": pack_vec(bin_), "bhn": pack_vec(bhn),
        "ones_row": np.ones((1, P), BF16_NP),
        "ones_col": np.ones((P, 1), np.float32),
    }

    def pack_act(a, rows):
        # [BL, D] slice -> per-bc [P, KT, NF] with [p, k, f] = a[bc*NF+f, k*P+p]
        arr = np.asarray(a, np.float32)[rows].T.reshape(KT, P, NB, NF)
        arr = arr.transpose(1, 0, 2, 3).astype(BF16_NP)
        return [np.ascontiguousarray(arr[:, :, bc, :]) for bc in range(NB)]

    in_maps = []
    for c in range(NCORES):
        rows = slice(c * BL, (c + 1) * BL)
        xp = pack_act(x, rows)
        hp = pack_act(h, rows)
        in_maps.append({
            **common,
            "x0": xp[0], "x1": xp[1], "h0": hp[0], "h1": hp[1],
        })

    res = bass_utils.run_bass_kernel_spmd(nc, in_maps,
                                          core_ids=list(range(NCORES)),
                                          trace=TRACE)
    LAST_RES = res
    new_hT = np.concatenate(
        [res.results[c]["nhT"].astype(np.float32) for c in range(NCORES)],
        axis=1)
    outT = np.concatenate(
        [res.results[c]["outT"].astype(np.float32) for c in range(NCORES)],
        axis=1)
    return np.ascontiguousarray(new_hT.T), np.ascontiguousarray(outT.T)


# revision 15
# speedup vs baseline: 1.2167x; 1.0153x over previous
"""GRUCell + LayerNorm readout fused Bass kernel for Trainium2 (8 NeuronCores).

Problem: B=8192, D=H=O=1024 fp32.
    r = sigmoid(x@Wir + bir + h@Whr)
    z = sigmoid(x@Wiz + biz + h@Whz)
    n = tanh(x@Win + bin_ + r*(h@Whn + bhn))
    new_h = (1-z)*n + z*h
    out = (LayerNorm(new_h)*ln_scale + ln_bias) @ Wout + bout

Strategy (v2):
  - Data-parallel over batch: core c gets rows [c*1024, (c+1)*1024); weights
    replicated, SBUF-resident in bf16 (loaded once, used for both batch
    chunks). No collectives.
  - Transposed domain: activations live as [feature, batch]; weights are the
    stationary operand in natural [k, h] layout; per-h gate biases become
    per-partition activation biases.
  - All matmul operands bf16 (measured end-to-end rel err ~7e-3 vs the 2e-2
    gate); PSUM + epilogue arithmetic fp32. Host pre-packs weights/x/h into
    the exact SBUF layouts so every DMA is 128 descriptors of contiguous
    >=1KB lines (enqueue- and HBM-efficient).
  - HAM pre-warm: a run of dummy matmuls on a memset tile at kernel start
    flips the PE clock gate to 8/8 before the first real matmul arrives.
  - Batch-split phasing hides the gates->readout boundary: phase A = gates
    for batch chunk 0; phase B = gates for chunk 1 interleaved with the
    readout for chunk 0; phase C = readout for chunk 1. The PE never waits
    on an epilogue chain.
  - LayerNorm folded into the readout:
        out = rstd[b]*( new_h@WoutF - mu[b]*colsum[o] + boutF[o]*sd[b] )
      with WoutF = ln_scale[:,None]*Wout, colsum = ln_scale@Wout,
      boutF = bout + ln_bias@Wout, sd[b] = sqrt(var+eps) = 1/rstd[b].
    The correction is a single K=2 rank-2 matmul into the same PSUM
    accumulator (stationary = [colsum; boutF], moving = [-mu; sd]), so the
    epilogue per readout tile is ONE vector op: out = po * rstd_bcast.
  - LN stats: per-tile elementwise accumulation of sum / sum-of-squares on
    DVE, one ones-column matmul per stat to reduce over h (partition dim),
    rstd broadcast back over partitions with a ones-row matmul.
"""

import sys
from contextlib import ExitStack

sys.path.insert(0, "/opt/trn_rl_repo")

import ml_dtypes
import numpy as np

import concourse.bacc as bacc
import concourse.mybir as mybir
import concourse.tile as tile
from concourse import bass_utils

B, D, H, O = 8192, 1024, 1024, 1024
NCORES = 8
BL = B // NCORES          # batch rows per core
P = 128                   # partitions
KT = D // P               # contraction tiles (8)
HT = H // P               # h output-partition tiles (8)
OT = O // P               # o output-partition tiles (8)
NB = 2                    # batch chunks per core
NF = BL // NB             # free dim per chunk (512)
LN_EPS = 1e-6
N_WARM = 96               # HAM pre-warm dummy matmuls

F32 = mybir.dt.float32
F32R = mybir.dt.float32r
BF16 = mybir.dt.bfloat16
BF16_NP = ml_dtypes.bfloat16

_COMPILED = None
TRACE = False
LAST_RES = None

XGATES = ("ir", "iz", "in")
HGATES = ("hr", "hz", "hn")
ALLGATES = XGATES + HGATES


def _build():
    nc = bacc.Bacc("TRN2", target_bir_lowering=False, debug=False,
                   num_devices=NCORES)
    sig = mybir.ActivationFunctionType.Sigmoid
    tanh = mybir.ActivationFunctionType.Tanh
    square = mybir.ActivationFunctionType.Square
    sqrtf = mybir.ActivationFunctionType.Sqrt
    copyf = mybir.ActivationFunctionType.Copy
    add_op = mybir.AluOpType.add
    sub_op = mybir.AluOpType.subtract
    mul_op = mybir.AluOpType.mult

    def din(name, shape, dt=BF16):
        return nc.dram_tensor(name, shape, dt, kind="ExternalInput").ap()

    def dout(name, shape, dt=BF16):
        return nc.dram_tensor(name, shape, dt, kind="ExternalOutput").ap()

    # host-pre-packed inputs (see kernel() for the exact layouts)
    x_d = [din(f"x{bc}", [P, KT, NF]) for bc in range(NB)]
    h_d = [din(f"h{bc}", [P, KT, NF]) for bc in range(NB)]
    w_d = {g: din(f"W{g}", [P, HT, KT * P]) for g in ALLGATES}
    woutF_d = din("woutF", [P, KT, O])
    colsum2_d = din("colsum2", [2, O])
    ones_row_d = din("ones_row", [1, P])
    ones_col_d = din("ones_col", [P, 1], F32R)
    bias_d = {v: din(v, [P, HT], F32) for v in ("bir", "biz", "bin", "bhn")}

    nhT_d = dout("nhT", [H, BL])
    outT_d = dout("outT", [O, BL])

    with tile.TileContext(nc) as tc, ExitStack() as ctx:
        singles = ctx.enter_context(tc.tile_pool(name="singles", bufs=1))
        gates = ctx.enter_context(tc.tile_pool(name="gates", bufs=1))
        ps = ctx.enter_context(tc.tile_pool(name="ps", bufs=1, space="PSUM"))

        # ---- HAM pre-warm: junk matmuls on a memset tile -------------------
        warm_sb = singles.tile([P, 64], BF16, name="warm_sb")
        nc.vector.memset(warm_sb[:], 0.0)
        eps_sb = singles.tile([1, 1], F32, name="eps_sb")
        nc.vector.memset(eps_sb[:], LN_EPS)
        pw = ps.tile([64, 64], F32, tag="r1", name="pw")
        for i in range(N_WARM):
            nc.tensor.matmul(pw[:], warm_sb[:], warm_sb[:],
                             start=True, stop=True)

        # ---- resident inputs, DMA-ordered to feed the PE ramp --------------
        x_sb = [singles.tile([P, KT, NF], BF16, name=f"x_sb{bc}")
                for bc in range(NB)]
        h_sb = [singles.tile([P, KT, NF], BF16, name=f"h_sb{bc}")
                for bc in range(NB)]
        w_sb = {g: singles.tile([P, HT, KT * P], BF16, name=f"w_{g}")
                for g in ALLGATES}
        woutF_sb = singles.tile([P, KT, O], BF16, name="woutF_sb")
        colsum2_sb = singles.tile([2, O], BF16, name="colsum2_sb")
        ones_row = singles.tile([1, P], BF16, name="ones_row")
        ones_col = singles.tile([P, 1], F32R, name="ones_col")
        bias_sb = {v: singles.tile([P, HT], F32, name=f"{v}_sb")
                   for v in ("bir", "biz", "bin", "bhn")}

        def load_w(g, ht):
            nc.sync.dma_start(w_sb[g][:, ht], w_d[g][:, ht])

        # supply order: ramp-critical first
        nc.sync.dma_start(x_sb[0][:, 0:2], x_d[0][:, 0:2])
        for g in XGATES:
            load_w(g, 0)
        nc.sync.dma_start(x_sb[0][:, 2:8], x_d[0][:, 2:8])
        for g in HGATES:
            load_w(g, 0)
        nc.sync.dma_start(h_sb[0][:, 0:4], h_d[0][:, 0:4])
        nc.sync.dma_start(h_sb[0][:, 4:8], h_d[0][:, 4:8])
        for v in ("bir", "biz", "bin", "bhn"):
            nc.sync.dma_start(bias_sb[v][:], bias_d[v])
        for g in ALLGATES:
            load_w(g, 1)
        for g in ALLGATES:
            load_w(g, 2)
        for g in ALLGATES:
            load_w(g, 3)
        nc.sync.dma_start(x_sb[1][:], x_d[1])
        nc.sync.dma_start(h_sb[1][:], h_d[1])
        for ht in range(4, HT):
            for g in ALLGATES:
                load_w(g, ht)
        nc.sync.dma_start(woutF_sb[:], woutF_d)
        nc.sync.dma_start(colsum2_sb[:], colsum2_d)
        nc.sync.dma_start(ones_row[:], ones_row_d)
        nc.sync.dma_start(ones_col[:], ones_col_d)

        # ---- persistent activations ---------------------------------------
        nh_sb = singles.tile([P, HT, BL], BF16, name="nh_sb")
        s_acc = [singles.tile([P, NF], F32R, name=f"s_acc{bc}")
                 for bc in range(NB)]
        q_acc = [singles.tile([P, NF], F32R, name=f"q_acc{bc}")
                 for bc in range(NB)]
        # [-mu ; sd] moving operand for the readout correction matmul
        mv = [singles.tile([2, NF], BF16, name=f"mv{bc}") for bc in range(NB)]
        rstd_f32 = [singles.tile([1, NF], F32, name=f"rstd_f32_{bc}")
                    for bc in range(NB)]
        rstd_row = [singles.tile([1, NF], BF16, name=f"rstd_row{bc}")
                    for bc in range(NB)]

        bsl = [slice(bc * NF, (bc + 1) * NF) for bc in range(NB)]

        # ---- gate group: 48 matmuls + epilogue ----------------------------
        gate_tags = {0: ("r0", "z0", "gi0", "gh0"), 1: ("r1", "z1", "gi1", "gh1")}

        def emit_gate_mms_x(ht, bc):
            tr, tz, tgi, _ = gate_tags[bc]
            pr = ps.tile([P, NF], F32, tag=tr, name=f"pr{bc}_{ht}")
            pz = ps.tile([P, NF], F32, tag=tz, name=f"pz{bc}_{ht}")
            pgi = ps.tile([P, NF], F32, tag=tgi, name=f"pgi{bc}_{ht}")
            for k in range(KT):
                ks = slice(k * P, (k + 1) * P)
                xs = x_sb[bc][:, k, :]
                nc.tensor.matmul(pr[:], w_sb["ir"][:, ht, ks], xs,
                                 start=(k == 0), stop=False)
                nc.tensor.matmul(pz[:], w_sb["iz"][:, ht, ks], xs,
                                 start=(k == 0), stop=False)
                nc.tensor.matmul(pgi[:], w_sb["in"][:, ht, ks], xs,
                                 start=(k == 0), stop=(k == KT - 1))
            # t3 = pgi + bin frees the gi bank well before the next group
            t3 = gates.tile([P, NF], F32, tag="t3", name=f"t3_{ht}_{bc}")
            nc.vector.tensor_scalar(t3[:], pgi[:], bias_sb["bin"][:, ht:ht + 1],
                                    None, add_op)
            return pr, pz, t3

        def emit_gate_mms_h(ht, bc):
            # gate-major so pr/pz stop early -> their banks free before the
            # next group's first matmuls need them
            tr, tz, tgi, tgh = gate_tags[bc]
            pr = ps.tile([P, NF], F32, tag=tr, name=f"prh{bc}_{ht}")
            pz = ps.tile([P, NF], F32, tag=tz, name=f"pzh{bc}_{ht}")
            pgh = ps.tile([P, NF], F32, tag=tgh, name=f"pgh{bc}_{ht}")
            for k in range(KT):
                nc.tensor.matmul(pr[:], w_sb["hr"][:, ht, k * P:(k + 1) * P],
                                 h_sb[bc][:, k, :],
                                 start=False, stop=(k == KT - 1))
            for k in range(KT):
                nc.tensor.matmul(pz[:], w_sb["hz"][:, ht, k * P:(k + 1) * P],
                                 h_sb[bc][:, k, :],
                                 start=False, stop=(k == KT - 1))
            for k in range(KT):
                nc.tensor.matmul(pgh[:], w_sb["hn"][:, ht, k * P:(k + 1) * P],
                                 h_sb[bc][:, k, :],
                                 start=(k == 0), stop=(k == KT - 1))
            return pr, pz, pgh

        last_parts = {}

        def emit_gate_epilogue(ht, bc, pr, pz, t3, pgh, last=False):
            hs = slice(ht * P, (ht + 1) * P)
            bs = bsl[bc]
            r_sb = gates.tile([P, NF], F32, tag="r_act", name=f"r_{ht}_{bc}")
            nc.scalar.activation(r_sb[:], pr[:], sig,
                                 bias=bias_sb["bir"][:, ht:ht + 1])
            z_sb = gates.tile([P, NF], F32, tag="z_act", name=f"z_{ht}_{bc}")
            nc.scalar.activation(z_sb[:], pz[:], sig,
                                 bias=bias_sb["biz"][:, ht:ht + 1])
            # h upcast for the blend (exact; off the DVE critical path)
            hf = gates.tile([P, NF], F32, tag=f"hf{ht % 2}", name=f"hf_{ht}_{bc}")
            nc.scalar.activation(hf[:], h_sb[bc][:, ht, :], copyf)

            # t = (pgh + bhn) * r ; t2 = t3 + t ; n = tanh(t2)
            t_sb = gates.tile([P, NF], F32, tag="t", name=f"t_{ht}_{bc}")
            nc.vector.scalar_tensor_tensor(
                t_sb[:], pgh[:], bias_sb["bhn"][:, ht:ht + 1], r_sb[:],
                add_op, mul_op)
            t2_sb = gates.tile([P, NF], F32, tag="u", name=f"t2_{ht}_{bc}")
            nc.vector.tensor_tensor(t2_sb[:], t3[:], t_sb[:], add_op)
            n_sb = gates.tile([P, NF], F32, tag="r_act", name=f"n_{ht}_{bc}")
            nc.scalar.activation(n_sb[:], t2_sb[:], tanh)

            # new_h = n + z*(h - n)
            u_sb = gates.tile([P, NF], F32, tag="t", name=f"u_{ht}_{bc}")
            nc.vector.tensor_tensor(u_sb[:], hf[:], n_sb[:], sub_op)
            v_sb = gates.tile([P, NF], F32, tag="v", name=f"v_{ht}_{bc}")
            nc.vector.tensor_mul(v_sb[:], z_sb[:], u_sb[:])

            if last:
                # final group feeds the stat matmuls directly (f32r moving)
                nhf = gates.tile([P, NF], F32R, tag="u", name=f"nhf_{ht}_{bc}")
                nc.vector.tensor_tensor(nhf[:], n_sb[:], v_sb[:], add_op)
                sq = gates.tile([P, NF], F32R, tag="t", name=f"sq_{ht}_{bc}")
                nc.scalar.activation(sq[:], nhf[:].bitcast(F32), square)
                last_parts[bc] = (nhf, sq)
                nc.scalar.activation(nh_sb[:, ht, bs], nhf[:].bitcast(F32),
                                     copyf)
                nc.gpsimd.dma_start(nhT_d[hs, bs], nh_sb[:, ht, bs])
                return

            nhf = gates.tile([P, NF], F32, tag="u", name=f"nhf_{ht}_{bc}")
            nc.vector.tensor_add(nhf[:], n_sb[:], v_sb[:])

            # LN stat partials (f32 accumulate over ht)
            if ht == 0:
                nc.vector.tensor_copy(s_acc[bc][:], nhf[:])
                nc.scalar.activation(q_acc[bc][:], nhf[:], square)
            else:
                nc.vector.tensor_tensor(s_acc[bc][:], s_acc[bc][:].bitcast(F32),
                                        nhf[:], add_op)
                sq = gates.tile([P, NF], F32, tag="t", name=f"sq_{ht}_{bc}")
                nc.scalar.activation(sq[:], nhf[:], square)
                nc.vector.tensor_tensor(q_acc[bc][:], q_acc[bc][:].bitcast(F32),
                                        sq[:], add_op)

            # bf16 copy feeds the readout matmul + the nhT store
            nc.scalar.activation(nh_sb[:, ht, bs], nhf[:], copyf)
            nc.gpsimd.dma_start(nhT_d[hs, bs], nh_sb[:, ht, bs])

        def emit_gate_group(ht, bc, last=False):
            pr, pz, t3 = emit_gate_mms_x(ht, bc)
            prh, pzh, pgh = emit_gate_mms_h(ht, bc)
            emit_gate_epilogue(ht, bc, prh, pzh, t3, pgh, last=last)

        # ---- LN stats: reduce + scale-factor chain ------------------------
        st_tags = {0: "gh0", 1: "z0"}
        st_q_tags = {0: "r0", 1: "r0"}
        pb_tags = {0: "gh0", 1: "gi0"}
        pb_ps = {}

        def emit_stat_mms(bc):
            # matmul PSUM dsts must start at partition 0 -> separate banks
            extra = last_parts.get(bc)
            st_s = ps.tile([1, NF], F32, tag=st_tags[bc], name=f"st_s{bc}")
            nc.tensor.matmul(st_s[:], ones_col[:], s_acc[bc][:],
                             start=True, stop=(extra is None))
            if extra is not None:
                nc.tensor.matmul(st_s[:], ones_col[:], extra[0][:],
                                 start=False, stop=True)
            st_q = ps.tile([1, NF], F32, tag=st_q_tags[bc], name=f"st_q{bc}")
            nc.tensor.matmul(st_q[:], ones_col[:], q_acc[bc][:],
                             start=True, stop=(extra is None))
            if extra is not None:
                nc.tensor.matmul(st_q[:], ones_col[:], extra[1][:],
                                 start=False, stop=True)
            return st_s, st_q

        def emit_stat_chain(bc, st):
            st_s, st_q = st
            # mv[0] = -mu (bf16) ; also f32 for mu^2
            nmu_f = gates.tile([1, NF], F32, tag="row0", name=f"nmu_f{bc}")
            nc.vector.tensor_scalar_mul(nmu_f[:], st_s[:], -1.0 / H)
            nc.vector.tensor_copy(mv[bc][0:1, :], nmu_f[:])
            mu2 = gates.tile([1, NF], F32, tag="row1", name=f"mu2_{bc}")
            nc.vector.tensor_mul(mu2[:], nmu_f[:], nmu_f[:])
            var = gates.tile([1, NF], F32, tag="row0", name=f"var_{bc}")
            nc.vector.scalar_tensor_tensor(var[:], st_q[:], 1.0 / H,
                                           mu2[:], mul_op, sub_op)
            # sd = sqrt(var + eps) -> mv[1] (bf16) and f32 for reciprocal
            sd_f = gates.tile([1, NF], F32, tag="row1", name=f"sd_f{bc}")
            nc.scalar.activation(sd_f[:], var[:], sqrtf, bias=eps_sb[:])
            # compute engines can't target partition 1; DMA the sd row there
            sd_bf = gates.tile([1, NF], BF16, tag="row2", name=f"sd_bf{bc}")
            nc.scalar.activation(sd_bf[:], sd_f[:], copyf)
            nc.gpsimd.dma_start(mv[bc][1:2, :], sd_bf[:])
            nc.vector.reciprocal(rstd_f32[bc][:], sd_f[:])
            nc.scalar.activation(rstd_row[bc][:], rstd_f32[bc][:], copyf)

        rstd_bc = [singles.tile([P, NF], F32, name=f"rstd_bc{bc}")
                   for bc in range(NB)]

        def emit_pb(bc):
            # DVE can read only one PSUM operand -> land the broadcast in SBUF
            pb = ps.tile([P, NF], F32, tag=pb_tags[bc], name=f"pb{bc}")
            nc.tensor.matmul(pb[:], ones_row[:], rstd_row[bc][:],
                             start=True, stop=True)
            nc.vector.tensor_copy(rstd_bc[bc][:], pb[:])
            pb_ps[bc] = rstd_bc[bc]

        # ---- readout group: 8 k-matmuls + rank-2 correction + 1 DVE op ----
        po_tags = {0: ("z0", "gi0", "r0"), 1: ("r1", "z1", "gi1", "gh1")}

        def emit_readout_mms(ot, bc):
            tags = po_tags[bc]
            po = ps.tile([P, NF], F32, tag=tags[ot % len(tags)],
                         name=f"po_{ot}_{bc}")
            os_ = slice(ot * P, (ot + 1) * P)
            bs = bsl[bc]
            for k in range(HT):
                nc.tensor.matmul(po[:], woutF_sb[:, k, os_],
                                 nh_sb[:, k, bs],
                                 start=(k == 0), stop=False)
            return po

        def emit_readout_corr(ot, bc, po):
            os_ = slice(ot * P, (ot + 1) * P)
            nc.tensor.matmul(po[:], colsum2_sb[:, os_], mv[bc][:],
                             start=False, stop=True)

        def emit_readout_fin(ot, bc, po):
            os_ = slice(ot * P, (ot + 1) * P)
            bs = bsl[bc]
            o_sb = gates.tile([P, NF], BF16, tag=f"o{ot % 3}",
                              name=f"o_{ot}_{bc}")
            nc.vector.tensor_mul(o_sb[:], po[:], pb_ps[bc][:])
            nc.scalar.dma_start(outT_d[os_, bs], o_sb[:])

        def emit_readout(ot, bc):
            po = emit_readout_mms(ot, bc)
            emit_readout_corr(ot, bc, po)
            emit_readout_fin(ot, bc, po)
            return po

        # ---- phase A: gates bc0 -------------------------------------------
        for ht in range(HT):
            emit_gate_group(ht, 0)

        # ---- phase B: gates bc1 + readout bc0 -----------------------------
        emit_gate_group(0, 1)
        st0 = emit_stat_mms(0)
        emit_stat_chain(0, st0)
        emit_gate_group(1, 1)
        po0 = emit_readout_mms(0, 0)
        emit_readout_corr(0, 0, po0)
        g2x = emit_gate_mms_x(2, 1)
        emit_pb(0)
        emit_readout_fin(0, 0, po0)
        g2h = emit_gate_mms_h(2, 1)
        emit_gate_epilogue(2, 1, g2h[0], g2h[1], g2x[2], g2h[2])
        emit_readout(1, 0)
        for ht in range(3, HT):
            emit_gate_group(ht, 1, last=(ht == HT - 1))
            if ht < HT - 1:
                emit_readout(ht - 1, 0)
        for ot in (5, 6, 7):
            emit_readout(ot, 0)

        # ---- phase C: readout bc1 -----------------------------------------
        st1 = emit_stat_mms(1)
        emit_stat_chain(1, st1)
        pos = {}
        for ot in range(OT):
            pos[ot] = emit_readout_mms(ot, 1)
            if ot == 2:
                emit_pb(1)
            if ot == 2:
                emit_readout_corr(0, 1, pos[0])
                emit_readout_fin(0, 1, pos.pop(0))
                emit_readout_corr(1, 1, pos[1])
                emit_readout_fin(1, 1, pos.pop(1))
            elif ot >= 3:
                emit_readout_corr(ot - 1, 1, pos[ot - 1])
                emit_readout_fin(ot - 1, 1, pos.pop(ot - 1))
        emit_readout_corr(7, 1, pos[7])
        emit_readout_fin(7, 1, pos.pop(7))

    nc.compile()
    return nc


def _pack_weight(w):
    # [D, H] -> [P, HT, KT*P] with [p, ht, k*P+j] = w[k*P+p, ht*P+j]
    t = np.asarray(w, np.float32).reshape(KT, P, HT, P)
    return np.ascontiguousarray(
        t.transpose(1, 2, 0, 3).reshape(P, HT, KT * P).astype(BF16_NP))


def kernel(x, h, Wir, bir, Wiz, biz, Win, bin_, Whr, Whz, Whn, bhn,
           ln_scale, ln_bias, Wout, bout):
    global _COMPILED, LAST_RES
    if _COMPILED is None:
        _COMPILED = _build()
    nc = _COMPILED

    ln_scale = np.asarray(ln_scale, np.float32)
    ln_bias = np.asarray(ln_bias, np.float32)
    Wout = np.asarray(Wout, np.float32)
    woutF = ln_scale[:, None] * Wout
    woutF_p = np.ascontiguousarray(
        woutF.reshape(KT, P, O).transpose(1, 0, 2).astype(BF16_NP))
    boutF = np.asarray(bout, np.float32) + ln_bias @ Wout
    colsum = ln_scale @ Wout
    colsum2 = np.ascontiguousarray(
        np.stack([colsum, boutF]).astype(BF16_NP))

    def pack_vec(v):
        return np.ascontiguousarray(
            np.asarray(v, np.float32).reshape(HT, P).T)

    common = {
        "Wir": _pack_weight(Wir), "Wiz": _pack_weight(Wiz),
        "Win": _pack_weight(Win), "Whr": _pack_weight(Whr),
        "Whz": _pack_weight(Whz), "Whn": _pack_weight(Whn),
        "woutF": woutF_p, "colsum2": colsum2,
        "bir": pack_vec(bir), "biz": pack_vec(biz),
        "bin": pack_vec(bin_), "bhn": pack_vec(bhn),
        "ones_row": np.ones((1, P), BF16_NP),
        "ones_col": np.ones((P, 1), np.float32),
    }

    def pack_act(a, rows):
        # [BL, D] slice -> per-bc [P, KT, NF] with [p, k, f] = a[bc*NF+f, k*P+p]
        arr = np.asarray(a, np.float32)[rows].T.reshape(KT, P, NB, NF)
        arr = arr.transpose(1, 0, 2, 3).astype(BF16_NP)
        return [np.ascontiguousarray(arr[:, :, bc, :]) for bc in range(NB)]

    in_maps = []
    for c in range(NCORES):
        rows = slice(c * BL, (c + 1) * BL)
        xp = pack_act(x, rows)
        hp = pack_act(h, rows)
        in_maps.append({
            **common,
            "x0": xp[0], "x1": xp[1], "h0": hp[0], "h1": hp[1],
        })

    res = bass_utils.run_bass_kernel_spmd(nc, in_maps,
                                          core_ids=list(range(NCORES)),
                                          trace=TRACE)
    LAST_RES = res
    new_hT = np.concatenate(
        [res.results[c]["nhT"].astype(np.float32) for c in range(NCORES)],
        axis=1)
    outT = np.concatenate(
        [res.results[c]["outT"].astype(np.float32) for c in range(NCORES)],
        axis=1)
    return np.ascontiguousarray(new_hT.T), np.ascontiguousarray(outT.T)
